# revision 1
# baseline (speedup 1.0000x reference)
"""3-layer GAT on Trainium2, 8-core SPMD Bass kernel.

Strategy (graph/data parallel, per sharding hint):
  - dst-nodes partitioned across 8 cores (1250 each, padded to 1280).
  - Dense phase (h = X @ [W | v_src | v_dst]) replicated on every core over all
    nodes -> per-core DRAM tables (h rows bf16, a_s/a_d f32).
  - Edge phase per core: dma_gather of h[src] / a_s[src] / a_d[dst] rows,
    exp(leaky_relu(a_s+a_d)) edge weights, then per-128-edge chunks a one-hot
    (edge -> dst-in-tile) matmul on PE accumulates weighted features + the
    softmax denominator into PSUM per dst tile.
  - Epilogue: normalize, +bias, ELU, BatchNorm -> bf16 X block; AllGather X
    across cores for the next layer. Final layer does log_softmax on chip.
"""

import math
import os
import sys

sys.path.insert(0, "/opt/trn_rl_repo")

import numpy as np
import ml_dtypes

P = 128
CORES = 8
HEADS, HID = 4, 64
FH = HEADS * HID  # 256
F2 = 64
EPS_BN = 1e-5
NEG_SLOPE = 0.2
BLK = int(__import__("os").environ.get("GAT_BLK", "8"))  # chunks per dma_gather block

BF16 = ml_dtypes.bfloat16


def _cdiv(a, b):
    return -(-a // b)


# ---------------------------------------------------------------------------
# host-side preprocessing of the edge structure
# ---------------------------------------------------------------------------

def preprocess_edges(n_nodes, edge_index):
    N = n_nodes
    NLOC = N // CORES
    NPAD = _cdiv(NLOC, P) * P
    NT = NPAD // P
    PN = NPAD * CORES

    loops = np.arange(N, dtype=np.int64)
    src = np.concatenate([edge_index[0].astype(np.int64), loops])
    dst = np.concatenate([edge_index[1].astype(np.int64), loops])
    order = np.argsort(dst, kind="stable")
    src_s, dst_s = src[order], dst[order]

    core_of = dst_s // NLOC
    lt = dst_s - core_of * NLOC          # local dst id on its core
    tile_of = lt // P
    dcol_of = (lt - tile_of * P).astype(np.float64)

    cnt = np.zeros((CORES, NT), np.int64)
    np.add.at(cnt, (core_of, tile_of), 1)
    # chunks per dst-tile, common across cores (SPMD: identical programs)
    chN = np.maximum(1, _cdiv(cnt, P).max(axis=0))
    c0 = np.concatenate([[0], np.cumsum(chN)]).astype(np.int64)
    J = int(c0[-1])
    EPAD = J * P

    # padded node id (node tables are [PN, *], row c*NPAD + local)
    pid_src = (src_s // NLOC) * NPAD + (src_s % NLOC)
    pid_dst = core_of * NPAD + lt

    isrc = np.zeros((CORES, EPAD), np.int64)
    idst = np.zeros((CORES, EPAD), np.int64)
    dcol = np.full((CORES, EPAD), 255.0, np.float32)
    for c in range(CORES):
        m = core_of == c
        t_c = tile_of[m]
        cc = np.concatenate([[0], np.cumsum(cnt[c])])
        k = np.arange(t_c.size) - cc[t_c]      # position within tile segment
        slot = c0[t_c] * P + k
        isrc[c, slot] = pid_src[m]
        idst[c, slot] = pid_dst[m]
        dcol[c, slot] = dcol_of[m]

    def wrap_idx(a):  # [EPAD] -> [128, EPAD//16] int16 (dma_gather layout)
        w = a.reshape(-1, 16).T.astype(np.int16)
        return np.ascontiguousarray(np.tile(w, (8, 1)))

    def wrap_dcol(a):  # [EPAD] -> [128, J] f32, slot j*128+p -> [p, j]
        return np.ascontiguousarray(a.reshape(J, P).T)

    # chunk -> (tile, is_first, is_last)
    chunk_info = []
    for t in range(NT):
        for j in range(int(c0[t]), int(c0[t + 1])):
            chunk_info.append((t, j == c0[t], j == c0[t + 1] - 1))
    rows = [min(P, NLOC - t * P) for t in range(NT)]

    cfg = dict(N=N, NLOC=NLOC, NPAD=NPAD, NT=NT, PN=PN, J=J, EPAD=EPAD,
               S=EPAD // 16, chunk_info=chunk_info, rows=rows)
    data = dict(
        isrc=[wrap_idx(isrc[c]) for c in range(CORES)],
        idst=[wrap_idx(idst[c]) for c in range(CORES)],
        dcol=[wrap_dcol(dcol[c]) for c in range(CORES)],
    )
    return cfg, data


def pack_consts(cfg, inp):
    """Shared (replicated) constant arrays for every core."""
    PN = cfg["PN"]
    N, NLOC, NPAD = cfg["N"], cfg["NLOC"], cfg["NPAD"]

    def wext(W, att_s, att_d, heads, out_c, ncols):
        fin = W.shape[0]
        w = np.zeros((fin, ncols), np.float32)
        w[:, : heads * out_c] = W
        for h in range(heads):
            blk = W[:, h * out_c : (h + 1) * out_c]
            w[:, heads * out_c + h] = blk @ att_s[h]
            w[:, heads * out_c + heads + h] = blk @ att_d[h]
        return w.astype(BF16)

    # x transposed & padded -> [F0, PN] bf16
    x = np.asarray(inp["x"], np.float32)
    F0 = x.shape[1]
    xt0 = np.zeros((F0, PN), np.float32)
    for c in range(CORES):
        blk = x[c * NLOC : (c + 1) * NLOC]
        xt0[:, c * NPAD : c * NPAD + NLOC] = blk.T
    consts = dict(xt0=xt0.astype(BF16))

    consts["w0"] = wext(np.asarray(inp["W0"], np.float32),
                        np.asarray(inp["att_src0"], np.float32),
                        np.asarray(inp["att_dst0"], np.float32), HEADS, HID, 384)
    consts["w1"] = wext(np.asarray(inp["W1"], np.float32),
                        np.asarray(inp["att_src1"], np.float32),
                        np.asarray(inp["att_dst1"], np.float32), HEADS, HID, 384)
    consts["w2"] = wext(np.asarray(inp["W2"], np.float32),
                        np.asarray(inp["att_src2"], np.float32),
                        np.asarray(inp["att_dst2"], np.float32), 1, F2, 128)

    def rep(v):
        return np.ascontiguousarray(np.tile(np.asarray(v, np.float32)[None, :], (P, 1)))

    for l, (g, be, m, v, b) in enumerate(
        [(inp["g0"], inp["be0"], inp["m0"], inp["v0"], inp["b0"]),
         (inp["g1"], inp["be1"], inp["m1"], inp["v1"], inp["b1"])]):
        A = np.asarray(g, np.float32) / np.sqrt(np.asarray(v, np.float32) + EPS_BN)
        B = np.asarray(be, np.float32) - np.asarray(m, np.float32) * A
        consts[f"bb{l}"] = rep(b)
        consts[f"aa{l}"] = rep(A)
        consts[f"cc{l}"] = rep(B)
    consts["bb2"] = rep(inp["b2"])
    consts["iota"] = np.ascontiguousarray(
        np.tile(np.arange(P, dtype=np.float32)[None, :], (P, 1)))
    return consts


# ---------------------------------------------------------------------------
# device program
# ---------------------------------------------------------------------------

def build_program(cfg, no_collectives=False, ablate="", repeat=1):
    import concourse.bacc as bacc
    import concourse.bass as bass
    import concourse.mybir as mybir
    import concourse.tile as tile

    f32 = mybir.dt.float32
    bf16 = mybir.dt.bfloat16
    i16 = mybir.dt.int16
    AF = mybir.ActivationFunctionType
    OP = mybir.AluOpType

    NLOC, NPAD, NT, PN, J, S = (cfg["NLOC"], cfg["NPAD"], cfg["NT"],
                                cfg["PN"], cfg["J"], cfg["S"])
    chunk_info = cfg["chunk_info"]
    rows = cfg["rows"]
    NBLK = _cdiv(J, BLK)
    F0 = 128

    nc = bacc.Bacc("TRN2", target_bir_lowering=False, debug=False,
                   num_devices=CORES)

    di = {}
    def dram_in(name, shape, dt):
        di[name] = nc.dram_tensor(name, list(shape), dt, kind="ExternalInput")
        return di[name]

    t_xt0 = dram_in("xt0", [F0, PN], bf16)
    t_isrc = dram_in("isrc", [P, S], i16)
    t_idst = dram_in("idst", [P, S], i16)
    t_dcol = dram_in("dcol", [P, J], f32)
    t_w0 = dram_in("w0", [F0, 384], bf16)
    t_w1 = dram_in("w1", [FH, 384], bf16)
    t_w2 = dram_in("w2", [FH, 128], bf16)
    t_bb0 = dram_in("bb0", [P, FH], f32)
    t_aa0 = dram_in("aa0", [P, FH], f32)
    t_cc0 = dram_in("cc0", [P, FH], f32)
    t_bb1 = dram_in("bb1", [P, FH], f32)
    t_aa1 = dram_in("aa1", [P, FH], f32)
    t_cc1 = dram_in("cc1", [P, FH], f32)
    t_bb2 = dram_in("bb2", [P, F2], f32)
    t_iota = dram_in("iota", [P, P], f32)
    t_out = nc.dram_tensor("out", [NLOC, F2], f32, kind="ExternalOutput")

    with tile.TileContext(nc) as tc:
        with (
            tc.tile_pool(name="const", bufs=1) as cp,
            tc.tile_pool(name="dram", bufs=1, space="DRAM") as dp,
            tc.tile_pool(name="work", bufs=2) as sb,
            tc.tile_pool(name="gath", bufs=2) as gp,
            tc.tile_pool(name="psum", bufs=2, space="PSUM") as pp,
        ):
            # ---- constants into SBUF
            def load_const(t, shape, dt, name):
                s = cp.tile(list(shape), dt, name=name)
                nc.sync.dma_start(out=s[:], in_=t.ap())
                return s

            c_isrc = load_const(t_isrc, [P, S], i16, "c_isrc")
            c_idst = load_const(t_idst, [P, S], i16, "c_idst")
            c_dcol = load_const(t_dcol, [P, J], f32, "c_dcol")
            c_iota = load_const(t_iota, [P, P], f32, "c_iota")
            c_w0 = load_const(t_w0, [F0, 384], bf16, "c_w0")
            c_w1a = cp.tile([P, 384], bf16, name="c_w1a")
            nc.sync.dma_start(out=c_w1a[:], in_=t_w1.ap()[0:P, :])
            c_w1b = cp.tile([P, 384], bf16, name="c_w1b")
            nc.sync.dma_start(out=c_w1b[:], in_=t_w1.ap()[P:FH, :])
            c_w2a = cp.tile([P, 128], bf16, name="c_w2a")
            nc.sync.dma_start(out=c_w2a[:], in_=t_w2.ap()[0:P, :])
            c_w2b = cp.tile([P, 128], bf16, name="c_w2b")
            nc.sync.dma_start(out=c_w2b[:], in_=t_w2.ap()[P:FH, :])
            c_bb0 = load_const(t_bb0, [P, FH], f32, "c_bb0")
            c_aa0 = load_const(t_aa0, [P, FH], f32, "c_aa0")
            c_cc0 = load_const(t_cc0, [P, FH], f32, "c_cc0")
            c_bb1 = load_const(t_bb1, [P, FH], f32, "c_bb1")
            c_aa1 = load_const(t_aa1, [P, FH], f32, "c_aa1")
            c_cc1 = load_const(t_cc1, [P, FH], f32, "c_cc1")
            c_bb2 = load_const(t_bb2, [P, F2], f32, "c_bb2")

            # ---- DRAM scratch
            h_tab = dp.tile([PN, FH + P], bf16, name="h_tab")
            asad_tab = dp.tile([PN, 64], f32, name="asad_tab")
            h2_tab = dp.tile([PN, 128], f32, name="h2_tab")
            asad2_tab = dp.tile([PN, 64], f32, name="asad2_tab")
            xl = dp.tile([NPAD, FH], bf16, name="xl")
            xg0 = dp.tile([PN, FH], bf16, name="xg0", addr_space="Shared")
            xg1 = dp.tile([PN, FH], bf16, name="xg1", addr_space="Shared")

            # zero the local pad rows of xl once (gathered as garbage o.w.)
            if NPAD > NLOC:
                zt = sb.tile([P, FH], bf16, tag="zpad", name="zpad", bufs=1)
                nc.vector.memset(zt[:], 0.0)
                nc.sync.dma_start(out=xl[NLOC:NPAD, :], in_=zt[: NPAD - NLOC, :])

            # ----------------------------------------------------------------
            def dense_phase(layer, xg=None):
                """h/as/ad tables for all PN nodes (replicated on each core)."""
                if layer == 0:
                    xta = sb.tile([P, PN], bf16, tag="xta", name="xta0", bufs=1)
                    nc.sync.dma_start(out=xta[:], in_=t_xt0.ap())
                    lhs = [xta]
                else:
                    # one whole-matrix transpose per feature half: X [PN, 256]
                    # -> XT halves [128, PN] resident in SBUF
                    xta = sb.tile([P, PN], bf16, tag="xta", name=f"xta{layer}",
                                  bufs=1)
                    nc.sync.dma_start(out=xta[:], in_=xg[:, 0:P], transpose=True)
                    xtb = sb.tile([P, PN], bf16, tag="xtb", name=f"xtb{layer}",
                                  bufs=1)
                    nc.sync.dma_start(out=xtb[:], in_=xg[:, P:FH], transpose=True)
                    lhs = [xta, xtb]
                for nt in range(PN // P):
                    sl = slice(nt * P, (nt + 1) * P)
                    if layer == 0:
                        ws, ncol = [c_w0], 384
                    else:
                        ws = [c_w1a, c_w1b] if layer == 1 else [c_w2a, c_w2b]
                        ncol = 384 if layer == 1 else 128
                    lh = [x[:, sl] for x in lhs]
                    ps = pp.tile([P, ncol], f32, tag="dps", name="dps", bufs=3)
                    for ki, (l_t, w_t) in enumerate(zip(lh, ws)):
                        nc.tensor.matmul(out=ps[:], lhsT=l_t, rhs=w_t[:],
                                         start=(ki == 0), stop=(ki == len(lh) - 1))
                    if layer < 2:
                        hrow = sb.tile([P, FH + P], bf16, tag="hrow", name="hrow", bufs=8)
                        nc.vector.tensor_copy(out=hrow[:], in_=ps[:, 0 : FH + P])
                        nc.sync.dma_start(out=h_tab[sl, :], in_=hrow[:])
                        arow = sb.tile([P, 64], f32, tag="arow", name="arow", bufs=8)
                        nc.vector.tensor_copy(out=arow[:], in_=ps[:, FH : FH + 64])
                        nc.sync.dma_start(out=asad_tab[sl, :], in_=arow[:])
                    else:
                        hrow = sb.tile([P, 128], f32, tag="h2row", name="h2row", bufs=8)
                        nc.vector.tensor_copy(out=hrow[:], in_=ps[:, 0:128])
                        nc.sync.dma_start(out=h2_tab[sl, :], in_=hrow[:])
                        arow = sb.tile([P, 64], f32, tag="a2row", name="a2row", bufs=8)
                        nc.vector.tensor_copy(out=arow[:], in_=ps[:, F2:128])
                        nc.sync.dma_start(out=asad2_tab[sl, :], in_=arow[:])

            # ----------------------------------------------------------------
            def epilogue01(layer, t, ps):
                r = rows[t]
                bb, aa, ccn = ((c_bb0, c_aa0, c_cc0) if layer == 0
                               else (c_bb1, c_aa1, c_cc1))
                r4 = sb.tile([P, HEADS], f32, tag="r4", name="r4", bufs=2)
                nc.vector.reciprocal(out=r4[:r], in_=ps[:r, FH : FH + HEADS])
                on = sb.tile([P, FH], f32, tag="on", name="on", bufs=2)
                nc.vector.tensor_tensor(
                    out=on[:r].rearrange("p (h c) -> p h c", h=HEADS),
                    in0=ps[:r, 0:FH].rearrange("p (h c) -> p h c", h=HEADS),
                    in1=r4[:r, :, None].to_broadcast([r, HEADS, HID]),
                    op=OP.mult)
                t1 = sb.tile([P, FH], f32, tag="t1", name="t1", bufs=2)
                nc.vector.tensor_tensor(out=t1[:r], in0=on[:r], in1=bb[:r],
                                        op=OP.add)
                mm = sb.tile([P, FH], f32, tag="mmn", name="mmn", bufs=2)
                nc.vector.tensor_scalar_min(mm[:r], t1[:r], 0.0)
                ee = sb.tile([P, FH], f32, tag="een", name="een", bufs=2)
                nc.scalar.activation(out=ee[:r], in_=mm[:r], func=AF.Exp)
                em = sb.tile([P, FH], f32, tag="emn", name="emn", bufs=2)
                nc.vector.tensor_scalar_add(em[:r], ee[:r], -1.0)
                ff = sb.tile([P, FH], f32, tag="ffn", name="ffn", bufs=2)
                nc.vector.tensor_tensor(out=ff[:r], in0=t1[:r], in1=em[:r],
                                        op=OP.max)
                yy = sb.tile([P, FH], f32, tag="yyn", name="yyn", bufs=2)
                nc.vector.tensor_tensor(out=yy[:r], in0=ff[:r], in1=aa[:r],
                                        op=OP.mult)
                xn = sb.tile([P, FH], bf16, tag="xnn", name="xnn", bufs=2)
                nc.vector.tensor_tensor(out=xn[:r], in0=yy[:r], in1=ccn[:r],
                                        op=OP.add)
                nc.sync.dma_start(out=xl[t * P : t * P + r, :], in_=xn[:r])

            def epilogue2(t, ps):
                r = rows[t]
                r1 = sb.tile([P, 1], f32, tag="r1", name="r1", bufs=2)
                nc.vector.reciprocal(out=r1[:r], in_=ps[:r, F2 : F2 + 1])
                o64 = sb.tile([P, F2], f32, tag="o64", name="o64", bufs=2)
                nc.vector.tensor_scalar_mul(o64[:r], ps[:r, 0:F2], r1[:r])
                ob = sb.tile([P, F2], f32, tag="ob", name="ob", bufs=2)
                nc.vector.tensor_tensor(out=ob[:r], in0=o64[:r], in1=c_bb2[:r],
                                        op=OP.add)
                mx = sb.tile([P, 1], f32, tag="mx", name="mx", bufs=2)
                nc.vector.reduce_max(mx[:r], ob[:r], axis=mybir.AxisListType.X,
                                     negate=True)
                eo = sb.tile([P, F2], f32, tag="eo", name="eo", bufs=2)
                nc.scalar.activation(out=eo[:r], in_=ob[:r], func=AF.Exp,
                                     bias=mx[:r])
                sm = sb.tile([P, 1], f32, tag="sm", name="sm", bufs=2)
                nc.vector.reduce_sum(sm[:r], eo[:r], axis=mybir.AxisListType.X)
                ls = sb.tile([P, 1], f32, tag="ls", name="ls", bufs=2)
                nc.scalar.activation(out=ls[:r], in_=sm[:r], func=AF.Ln)
                fo = sb.tile([P, F2], f32, tag="fo", name="fo", bufs=2)
                nc.vector.tensor_scalar(out=fo[:r], in0=ob[:r], scalar1=mx[:r],
                                        scalar2=ls[:r], op0=OP.add,
                                        op1=OP.subtract)
                nc.sync.dma_start(out=t_out.ap()[t * P : t * P + r, :],
                                  in_=fo[:r])

            # ----------------------------------------------------------------
            def edge_phase(layer):
                if ablate == "dense":
                    return
                big = layer < 2
                ps_cur = None
                for b in range(NBLK):
                    j0 = b * BLK
                    jn = min(J, j0 + BLK) - j0
                    ne = jn * P
                    isl = slice(j0 * 8, (j0 + jn) * 8)
                    if big:
                        hg = gp.tile([P, jn, FH + P], bf16, tag="hg", name="hg", bufs=3)
                        adg = gp.tile([P, jn, 64], f32, tag="adg", name="adg", bufs=3)
                        if ablate != "chunk":
                            nc.gpsimd.dma_gather(hg[:], h_tab[:], c_isrc[:, isl],
                                                 ne, ne, FH + P)
                            nc.gpsimd.dma_gather(adg[:], asad_tab[:],
                                                 c_idst[:, isl], ne, ne, 64)
                        nh = HEADS
                        a_s = hg[:, :, FH : FH + HEADS]
                        a_d = adg[:, :, nh : 2 * nh]
                    else:
                        hg = gp.tile([P, jn, 128], f32, tag="hg2", name="hg2", bufs=3)
                        adg = gp.tile([P, jn, 64], f32, tag="adg2", name="adg2", bufs=3)
                        if ablate != "chunk":
                            nc.gpsimd.dma_gather(hg[:], h2_tab[:], c_isrc[:, isl],
                                                 ne, ne, 128)
                            nc.gpsimd.dma_gather(adg[:], asad2_tab[:],
                                                 c_idst[:, isl], ne, ne, 64)
                        nh = 1
                        a_s = hg[:, :, F2 : F2 + 1]
                        a_d = adg[:, :, 1:2]

                    asf = gp.tile([P, jn, nh], f32, tag=f"asf{nh}", name=f"asf{nh}", bufs=2)
                    nc.vector.tensor_copy(out=asf[:], in_=a_s)
                    zz = gp.tile([P, jn, nh], f32, tag=f"zz{nh}", name=f"zzt{nh}", bufs=2)
                    nc.vector.tensor_tensor(out=zz[:], in0=asf[:], in1=a_d,
                                            op=OP.add)
                    z2 = gp.tile([P, jn, nh], f32, tag=f"z2{nh}", bufs=2)
                    nc.vector.tensor_scalar_mul(z2[:], zz[:], NEG_SLOPE)
                    lr = gp.tile([P, jn, nh], f32, tag=f"lr{nh}", bufs=2)
                    nc.vector.tensor_tensor(out=lr[:], in0=zz[:], in1=z2[:],
                                            op=OP.max)
                    ew = gp.tile([P, jn, nh], bf16 if big else f32,
                                 tag=f"ew{nh}", bufs=2)
                    nc.scalar.activation(out=ew[:], in_=lr[:], func=AF.Exp)

                    for jj in range(jn if ablate != "gather" else 0):
                        j = j0 + jj
                        t, is_first, is_last = chunk_info[j]
                        if is_first:
                            ps_cur = pp.tile([P, FH + HEADS if big else F2 + 1],
                                             f32, tag="agg", name="agg" if big else "agg2",
                                             bufs=4)
                        if big:
                            oh = sb.tile([P, P], bf16, tag="oh", name="oh", bufs=6)
                            nc.vector.tensor_tensor(
                                out=oh[:],
                                in0=c_dcol[:, j : j + 1].to_broadcast([P, P]),
                                in1=c_iota[:], op=OP.is_equal)
                            rhs = sb.tile([P, FH + HEADS], bf16, tag="rhs", name="rhs",
                                          bufs=4)
                            nc.vector.tensor_tensor(
                                out=rhs[:, 0:FH].rearrange(
                                    "p (h c) -> p h c", h=HEADS),
                                in0=hg[:, jj, 0:FH].rearrange(
                                    "p (h c) -> p h c", h=HEADS),
                                in1=ew[:, jj, :, None].to_broadcast(
                                    [P, HEADS, HID]),
                                op=OP.mult)
                            nc.scalar.copy(out=rhs[:, FH : FH + HEADS],
                                           in_=ew[:, jj, :])
                        else:
                            oh = sb.tile([P, P], f32, tag="oh2", name="oh2", bufs=6)
                            nc.vector.tensor_tensor(
                                out=oh[:],
                                in0=c_dcol[:, j : j + 1].to_broadcast([P, P]),
                                in1=c_iota[:], op=OP.is_equal)
                            rhs = sb.tile([P, F2 + 1], f32, tag="rhs2", name="rhs2", bufs=6)
                            nc.vector.tensor_scalar_mul(
                                rhs[:, 0:F2], hg[:, jj, 0:F2], ew[:, jj, :])
                            nc.scalar.copy(out=rhs[:, F2 : F2 + 1],
                                           in_=ew[:, jj, :])
                        nc.tensor.matmul(out=ps_cur[:], lhsT=oh[:], rhs=rhs[:],
                                         start=is_first, stop=is_last)
                        if is_last:
                            if big:
                                epilogue01(layer, t, ps_cur)
                            else:
                                epilogue2(t, ps_cur)

            # ---------------- program ----------------
            rg = [list(range(CORES))]
            for _rep in range(repeat):
                if _rep > 0:
                    xg0 = dp.tile([PN, FH], bf16, name=f"xg0r{_rep}",
                                  addr_space="Shared")
                    xg1 = dp.tile([PN, FH], bf16, name=f"xg1r{_rep}",
                                  addr_space="Shared")
                dense_phase(0)
                edge_phase(0)
                if no_collectives:
                    nc.sync.dma_start(out=xg0[0:NPAD, :], in_=xl[:])
                else:
                    nc.gpsimd.collective_compute(
                        "AllGather", mybir.AluOpType.bypass, replica_groups=rg,
                        ins=[xl.opt()], outs=[xg0.opt()])
                dense_phase(1, xg0)
                edge_phase(1)
                if no_collectives:
                    nc.sync.dma_start(out=xg1[0:NPAD, :], in_=xl[:])
                else:
                    nc.gpsimd.collective_compute(
                        "AllGather", mybir.AluOpType.bypass, replica_groups=rg,
                        ins=[xl.opt()], outs=[xg1.opt()])
                dense_phase(2, xg1)
                edge_phase(2)

    nc.compile()
    return nc


# ---------------------------------------------------------------------------
# entry point
# ---------------------------------------------------------------------------

def make_in_maps(cfg, data, consts):
    in_maps = []
    for c in range(CORES):
        m = dict(consts)
        m["isrc"] = data["isrc"][c]
        m["idst"] = data["idst"][c]
        m["dcol"] = data["dcol"][c]
        in_maps.append(m)
    return in_maps


def kernel(**inputs):
    from concourse.bass_utils import run_bass_kernel_spmd

    x = np.asarray(inputs["x"])
    edge_index = np.asarray(inputs["edge_index"])
    cfg, data = preprocess_edges(x.shape[0], edge_index)
    consts = pack_consts(cfg, inputs)
    nc = build_program(cfg)
    in_maps = make_in_maps(cfg, data, consts)
    res = run_bass_kernel_spmd(nc, in_maps, core_ids=list(range(CORES)))
    NLOC = cfg["NLOC"]
    out = np.concatenate(
        [np.asarray(res.results[c]["out"], np.float32) for c in range(CORES)],
        axis=0)
    return out

